# revision 26
# baseline (speedup 1.0000x reference)
"""Trainium2 Bass kernel for nn_BlockAttnRes (fused RMSNorm-softmax pooling), v3.

Reference computation (all fp32):
    V = concat([blocks, partial[None]], axis=0)          # (8, B, T, D)
    K = V * rsqrt(mean(V^2, -1) + eps) * norm_weight
    logits  = einsum('d,nbtd->nbt', w, K)
    weights = softmax(logits, axis=0)                    # over the 8 sources
    out     = einsum('nbt,nbtd->btd', weights, V)        # (B, T, D)

v3 design (vs v2 at ~130 us/iter, DVE+ACT ~98 us each, DMA ~104 us):
  * The 2x/4x DVE & ACT perf modes never engage for accum-bearing ops
    (HW-measured: any accum_out forces 1x mode), so each [128,1024]
    stats chunk costs ~1.3-1.4 us regardless of engine/dtype. Pool
    can't take reductions (no accum codegen, cross-partition-only
    reduce), so stats stay DVE/ACT, balanced: DVE = 64 dots + a few
    squares, ACT = remaining squares.
  * rsqrt via exp(-0.5*ln(ms)): Ln and Exp share one ACT table set
    (natural_log_exp_and_others) and Square is filler in every set, so
    the ACT table never reloads (v2 paid 4 table switches/iter, 5 us).
  * Output stored as f16 (host upcasts): halves store traffic; HBM
    floor drops from 36 to 34 MiB/core.
  * PSUM evac on ACT Copy (f32->f16).

Sharding: data-parallel over the 8192 tokens (B*T) across 8 NeuronCores.
"""

import os
import sys

import numpy as np

sys.path.insert(0, "/opt/trn_rl_repo")

N_BLOCKS, B, T, D = 7, 4, 2048, 1024
N_SRC = N_BLOCKS + 1          # 8 sources after appending `partial`
N_CORES = 8
TOK_TOTAL = B * T             # 8192
TOK_PER_CORE = TOK_TOTAL // N_CORES   # 1024
P = 128                       # SBUF partitions
QUADS = int(os.environ.get("KERNEL_QUADS", "2"))  # 128-token tiles per group
VBUFS = int(os.environ.get("KERNEL_VBUFS", "3"))  # v-tile pool depth
EPS = float(np.finfo(np.float32).eps)

# engine-split knobs (env-overridable for benchmarking)
# (Pool cannot take stats chunks: STT/TS accum_out fails codegen on Pool and
# gpsimd tensor_reduce is cross-partition only. Stats split DVE/ACT two-way.)
N_SQ_DVE = int(os.environ.get("KERNEL_N_SQ_DVE", "0"))   # sq chunks/group -> DVE
N_FOLD = int(os.environ.get("KERNEL_N_FOLD", "0"))       # sources folded on DVE STT
DIAG_ENG = os.environ.get("KERNEL_DIAG_ENG", "gpsimd")   # 'dve' | 'gpsimd'
EVAC_ACT = int(os.environ.get("KERNEL_EVAC_ACT", "0"))   # quads/group evac'd on ACT
OUT_F16 = os.environ.get("KERNEL_OUT_F16", "1") == "1"
LNEXP = os.environ.get("KERNEL_LNEXP", "1") == "1"       # rsqrt via exp(-ln/2)
# PSUM sinks for dummy outs hang the exec unit (NRT_EXEC_UNIT_UNRECOVERABLE,
# likely the PSUM-out + SBUF-accum_out encoding); keep off.
SINK_PSUM = os.environ.get("KERNEL_SINK_PSUM", "0") == "1"

_STATE: dict = {}


def _split_multi_waits(nc):
    """TPB instructions encode a single sem-wait; this walrus build refuses
    instructions carrying more (`Too many sync wait commands`). Split extra
    waits onto single-wait NoOps on the same engine, preserving per-engine
    program order (and therefore semantics)."""
    import concourse.mybir as mybir

    for fn in nc.m.functions:
        for blk in fn.blocks:
            insts = list(blk.instructions)
            out = []
            changed = False
            for ins in insts:
                si = ins.sync_info
                if si is not None and len(si.on_wait) > 1:
                    waits = list(si.on_wait)
                    for k, w in enumerate(waits[:-1]):
                        nop = mybir.InstNoOp(name=f"{ins.name}-sw{k}", ins=[], outs=[])
                        nop.engine = ins.engine
                        nop.sync_info = mybir.SyncInfo(on_wait=[w], on_update=[])
                        out.append(nop)
                    ins.sync_info = mybir.SyncInfo(
                        on_wait=[waits[-1]], on_update=list(si.on_update)
                    )
                    changed = True
                out.append(ins)
            if changed:
                blk.instructions = out
    return nc


def _spread(k: int, n: int = 32) -> set:
    """k indices spread evenly over [0, n)."""
    return {(j * n) // k for j in range(k)} if k > 0 else set()


def _build_nc(
    repeat: int = 1,
    loop: bool = True,
    n_sq_dve: int = N_SQ_DVE,
    n_fold: int = N_FOLD,
    diag_eng: str = DIAG_ENG,
    evac_act: int = EVAC_ACT,
    out_f16: bool = OUT_F16,
    lnexp: bool = LNEXP,
    quads: int = QUADS,
    vbufs: int = VBUFS,
    sink_psum: bool = SINK_PSUM,
):
    import concourse.bass as bass
    import concourse.mybir as mybir
    import concourse.tile as tile
    from contextlib import ExitStack

    f32 = mybir.dt.float32
    f16 = mybir.dt.float16
    Alu = mybir.AluOpType
    Act = mybir.ActivationFunctionType

    QUADS = quads                      # shadow module default inside builder
    N_G = TOK_PER_CORE // (P * QUADS)  # groups per core
    out_dt = f16 if out_f16 else f32

    nc = bass.Bass("TRN2", target_bir_lowering=False, debug=False)

    blocks_d = nc.dram_tensor(
        "blocks", [N_BLOCKS, TOK_PER_CORE, D], f32, kind="ExternalInput"
    )
    partial_d = nc.dram_tensor("partial", [TOK_PER_CORE, D], f32, kind="ExternalInput")
    wn_d = nc.dram_tensor("wnb", [P, D], f16, kind="ExternalInput")
    ident_d = nc.dram_tensor("identb", [P, 1, P], f16, kind="ExternalInput")
    out_d = nc.dram_tensor("out", [TOK_PER_CORE, D], out_dt, kind="ExternalOutput")

    # (g p q) d -> g p q d views: one DMA moves a full 2 MiB contiguous group
    # slice into a [128, 4, 1024] SBUF tile; each partition gets a contiguous
    # 16 KiB run (4 consecutive tokens), so descriptors are large and dense.
    # The in-group token permutation (partition-major) cancels between load
    # and store.
    bap = blocks_d.ap().rearrange("n (g p q) d -> n g p q d", p=P, q=QUADS)
    pap = partial_d.ap().rearrange("(g p q) d -> g p q d", p=P, q=QUADS)
    oap = out_d.ap().rearrange("(g p q) d -> g p q d", p=P, q=QUADS)

    fold_src = list(range(n_fold))                    # folded on DVE STT
    pe_src = list(range(n_fold, N_SRC))               # weighted sum on PE
    NQ8 = N_SRC * QUADS                               # stats width: 32

    dve_sq = _spread(n_sq_dve, N_SRC * QUADS)

    with tile.TileContext(nc) as tc, ExitStack() as ctx:
        const_pool = ctx.enter_context(tc.tile_pool(name="const", bufs=1))
        vpool = ctx.enter_context(tc.tile_pool(name="v", bufs=vbufs))
        scr_pool = ctx.enter_context(tc.tile_pool(name="scr", bufs=2))
        stat_pool = ctx.enter_context(tc.tile_pool(name="stat", bufs=2))
        diag_pool = ctx.enter_context(tc.tile_pool(name="diag", bufs=2))
        out_pool = ctx.enter_context(tc.tile_pool(name="outp", bufs=2))
        psum_pool = ctx.enter_context(tc.tile_pool(name="ps", bufs=1, space="PSUM"))

        wn_sb = const_pool.tile([P, D], f16, name="wn_sb")
        nc.sync.dma_start(wn_sb[:], wn_d.ap()[:, :])
        ident_sb = const_pool.tile([P, 1, P], f16, name="ident_sb")
        nc.sync.dma_start(ident_sb[:], ident_d.ap()[:, :, :])

        # PSUM sinks for the discarded elementwise outputs of accum ops:
        # keeps ~34 MB/iter of garbage writes off the SBUF ports (they were
        # slowing both the SDMA loads and the accum ops themselves).
        sinkV = sinkA = None
        if sink_psum:
            sinkV = psum_pool.tile([P, D], f32, tag="sinkV", name="sinkV", bufs=1)
            sinkA = psum_pool.tile([P, D], f32, tag="sinkA", name="sinkA", bufs=1)

        def emit_loads(g, r=0):
            # Loads get the most-negative priority: on the shared gpsimd
            # queue they must always outrank stats chunks and anything else,
            # so the SWDGE wire never sits behind a compute-dependent op in
            # the frozen per-engine order.
            v = []
            with tc.high_priority(offset=10**6):
                for n in range(N_BLOCKS):
                    vt = vpool.tile(
                        [P, QUADS, D], f16, tag=f"v{n}", name=f"v{n}_{g}_{r}"
                    )
                    nc.gpsimd.dma_start(vt[:], bap[n, g])
                    v.append(vt)
                vt = vpool.tile([P, QUADS, D], f16, tag="v7", name=f"v7_{g}_{r}")
                nc.gpsimd.dma_start(vt[:], pap[g])
                v.append(vt)
            return v

        def emit_stats_softmax(g, r, v):
            acc = out_pool.tile([P, QUADS, D], out_dt, tag="acc", name=f"acc_{g}_{r}")
            # ---- stats: s2 + dot, chunks balanced over ACT/DVE/Pool ----
            s2 = stat_pool.tile([P, NQ8], f32, tag="s2", name=f"s2_{g}")
            dot = stat_pool.tile([P, NQ8], f32, tag="dot", name=f"dot_{g}")
            if sink_psum:
                out_act, out_dve = sinkA[:], sinkV[:]
            else:
                dum_act = scr_pool.tile([P, 1], f16, tag="dumA", name=f"dumA_{g}")
                dum_dve = scr_pool.tile([P, 1], f16, tag="dumV", name=f"dumV_{g}")
                out_act = dum_act[:].broadcast_to((P, D))
                out_dve = dum_dve[:].broadcast_to((P, D))
            for n in range(N_SRC):
                # source-outer: the frozen per-engine order then consumes
                # sources in load-arrival order
                for q in range(QUADS):
                    i = n * QUADS + q
                    col = q * N_SRC + n
                    if i in dve_sq:
                        nc.vector.scalar_tensor_tensor(
                            out=out_dve,
                            in0=v[n][:, q, :],
                            scalar=1.0,
                            in1=v[n][:, q, :],
                            op0=Alu.mult,
                            op1=Alu.mult,
                            accum_out=s2[:, col : col + 1],
                        )
                    else:
                        nc.scalar.activation(
                            out_act,
                            v[n][:, q, :],
                            Act.Square,
                            accum_out=s2[:, col : col + 1],
                        )
                    nc.vector.scalar_tensor_tensor(
                        out=out_dve,
                        in0=v[n][:, q, :],
                        scalar=1.0,
                        in1=wn_sb[:],
                        op0=Alu.mult,
                        op1=Alu.mult,
                        accum_out=dot[:, col : col + 1],
                    )

            # ---- softmax over the 8 sources, batched over all quads ----
            # High priority (fixed offset): this chain must beat the NEXT
            # group's stats in the scheduler's per-engine heaps.
            hp = tc.high_priority()
            hp.__enter__()
            ms = stat_pool.tile([P, NQ8], f32, tag="ms", name=f"ms_{g}")
            nc.vector.tensor_scalar(ms[:], s2[:], 1.0 / D, EPS, Alu.mult, Alu.add)
            y = stat_pool.tile([P, NQ8], f32, tag="y", name=f"y_{g}")
            if lnexp:
                # y = 1/sqrt(ms) = exp(-0.5*ln(ms)): Ln+Exp live in ONE act
                # table set, so the ACT tables never reload across the run.
                t_ln = stat_pool.tile([P, NQ8], f32, tag="tln", name=f"tln_{g}")
                nc.scalar.activation(t_ln[:], ms[:], Act.Ln)
                nc.scalar.activation(y[:], t_ln[:], Act.Exp, scale=-0.5)
            else:
                rt = stat_pool.tile([P, NQ8], f32, tag="rt", name=f"rt_{g}")
                nc.scalar.activation(rt[:], ms[:], Act.Sqrt)
                nc.vector.reciprocal(y[:], rt[:])

            lg = stat_pool.tile([P, QUADS, N_SRC], f32, tag="lg", name=f"lg_{g}")
            nc.vector.tensor_tensor(
                lg[:].rearrange("p q n -> p (q n)"), dot[:], y[:], Alu.mult
            )
            nm = stat_pool.tile([P, QUADS, 1], f32, tag="nm", name=f"nm_{g}")
            nc.vector.tensor_reduce(
                nm[:], lg[:], axis=mybir.AxisListType.X, op=Alu.max, negate=True
            )
            lgs = stat_pool.tile([P, QUADS, N_SRC], f32, tag="lgs", name=f"lgs_{g}")
            nc.vector.tensor_tensor(
                lgs[:], lg[:], nm[:].broadcast_to((P, QUADS, N_SRC)), Alu.add
            )
            e = stat_pool.tile([P, QUADS, N_SRC], f32, tag="e", name=f"e_{g}")
            nc.scalar.activation(
                e[:].rearrange("p q n -> p (q n)"),
                lgs[:].rearrange("p q n -> p (q n)"),
                Act.Exp,
            )
            den = stat_pool.tile([P, QUADS, 1], f32, tag="den", name=f"den_{g}")
            nc.vector.tensor_reduce(den[:], e[:], axis=mybir.AxisListType.X, op=Alu.add)
            rcp = stat_pool.tile([P, QUADS, 1], f32, tag="rcp", name=f"rcp_{g}")
            nc.vector.reciprocal(rcp[:], den[:])
            wgt = stat_pool.tile([P, QUADS, N_SRC], f32, tag="wgt", name=f"wgt_{g}")
            nc.vector.tensor_tensor(
                wgt[:], e[:], rcp[:].broadcast_to((P, QUADS, N_SRC)), Alu.mult
            )
            return {"g": g, "r": r, "v": v, "wgt": wgt, "acc": acc, "hp": hp}

        def emit_wsum(st):
            g, r, v, wgt = st["g"], st["r"], st["v"], st["wgt"]
            acc, hp = st["acc"], st["hp"]

            # ---- weighted sum: PE diag matmuls + evac / DVE folds ----
            half = D // 2
            for q in range(QUADS):
                # all 8 diag matrices of this quad built in ONE fat op:
                # dga[p, n, j] = ident[p, j] * wgt[p, q, n]. Per-op overhead
                # made 32 tiny diag builds cost 28 us/group serial on Pool --
                # the one-shot build is ~1.3 us.
                dga = diag_pool.tile(
                    [P, N_SRC, P], f16, tag=f"dga{q}", name=f"dga{q}_{g}"
                )
                i_b = ident_sb[:].broadcast_to((P, N_SRC, P))
                w_b = wgt[:, q, :].broadcast_to((P, N_SRC, P))
                if diag_eng == "dve":
                    nc.vector.tensor_tensor(dga[:], i_b, w_b, Alu.mult)
                else:
                    nc.gpsimd.tensor_tensor(dga[:], i_b, w_b, Alu.mult)
                ps = psum_pool.tile(
                    [P, D], f32, tag=f"ps{q}", name=f"ps{q}_{g}",
                    bufs=(1 if (sink_psum or QUADS > 2) else 2),
                )
                for c in range(2):
                    cs = slice(c * half, (c + 1) * half)
                    for j, n in enumerate(pe_src):
                        nc.tensor.matmul(
                            ps[:, cs],
                            lhsT=dga[:, n, :],
                            rhs=v[n][:, q, cs],
                            start=(j == 0),
                            stop=(j == len(pe_src) - 1),
                        )
                if fold_src:
                    n0 = fold_src[0]
                    nc.vector.scalar_tensor_tensor(
                        out=acc[:, q, :],
                        in0=v[n0][:, q, :],
                        scalar=wgt[:, q, n0 : n0 + 1],
                        in1=ps[:],
                        op0=Alu.mult,
                        op1=Alu.add,
                    )
                    for n in fold_src[1:]:
                        nc.vector.scalar_tensor_tensor(
                            out=acc[:, q, :],
                            in0=v[n][:, q, :],
                            scalar=wgt[:, q, n : n + 1],
                            in1=acc[:, q, :],
                            op0=Alu.mult,
                            op1=Alu.add,
                        )
                elif q < evac_act:
                    nc.scalar.activation(acc[:, q, :], ps[:], Act.Copy)
                else:
                    nc.vector.tensor_scalar(
                        acc[:, q, :], ps[:], 1.0, 0.0, Alu.mult, Alu.add
                    )

            nc.sync.dma_start(oap[g], acc[:])
            hp.__exit__(None, None, None)

        def run_groups(rs):
            for r in rs:
                for g in range(N_G):
                    v = emit_loads(g, r)
                    emit_wsum(emit_stats_softmax(g, r, v))

        if repeat == 1 or not loop:
            run_groups(range(repeat))
        else:
            with tc.For_i(0, repeat, 1):
                run_groups([0])

    return _split_multi_waits(nc)


def _get_state():
    if "nc" not in _STATE:
        _STATE["nc"] = _build_nc()
    return _STATE["nc"]


def _prepare_in_maps(blocks, partial, norm_weight, w):
    blocks = np.asarray(blocks, dtype=np.float32)
    partial = np.asarray(partial, dtype=np.float32)
    norm_weight = np.asarray(norm_weight, dtype=np.float32)
    w = np.asarray(w, dtype=np.float32)

    wn = (w * norm_weight).astype(np.float32)
    wn_b = np.ascontiguousarray(np.broadcast_to(wn, (P, D)).astype(np.float16))
    ident = np.eye(P, dtype=np.float16).reshape(P, 1, P)

    blocks_f = blocks.reshape(N_BLOCKS, TOK_TOTAL, D)
    partial_f = partial.reshape(TOK_TOTAL, D)

    in_maps = []
    for c in range(N_CORES):
        sl = slice(c * TOK_PER_CORE, (c + 1) * TOK_PER_CORE)
        in_maps.append(
            {
                "blocks": np.ascontiguousarray(blocks_f[:, sl, :]),
                "partial": np.ascontiguousarray(partial_f[sl, :]),
                "wnb": wn_b,
                "identb": ident,
            }
        )
    return in_maps


def _run(inputs, trace=False, **kwargs):
    from concourse.bass_utils import run_bass_kernel_spmd

    nc = _get_state()
    in_maps = _prepare_in_maps(**inputs)
    bkr = run_bass_kernel_spmd(
        nc, in_maps, core_ids=list(range(N_CORES)), trace=trace, **kwargs
    )
    out = np.concatenate([bkr.results[c]["out"] for c in range(N_CORES)], axis=0)
    return out.reshape(B, T, D).astype(np.float32), bkr


def kernel(**inputs) -> np.ndarray:
    out, _ = _run(inputs, trace=False)
    return out


# revision 29
# speedup vs baseline: 1.0428x; 1.0428x over previous
"""Trainium2 Bass kernel for nn_BlockAttnRes (fused RMSNorm-softmax pooling), v3.

Reference computation (all fp32):
    V = concat([blocks, partial[None]], axis=0)          # (8, B, T, D)
    K = V * rsqrt(mean(V^2, -1) + eps) * norm_weight
    logits  = einsum('d,nbtd->nbt', w, K)
    weights = softmax(logits, axis=0)                    # over the 8 sources
    out     = einsum('nbt,nbtd->btd', weights, V)        # (B, T, D)

v3 design (vs v2 at ~130 us/iter, DVE+ACT ~98 us each, DMA ~104 us):
  * The 2x/4x DVE & ACT perf modes never engage for accum-bearing ops
    (HW-measured: any accum_out forces 1x mode), so each [128,1024]
    stats chunk costs ~1.3-1.4 us regardless of engine/dtype. Pool
    can't take reductions (no accum codegen, cross-partition-only
    reduce), so stats stay DVE/ACT, balanced: DVE = 64 dots + a few
    squares, ACT = remaining squares.
  * rsqrt via exp(-0.5*ln(ms)): Ln and Exp share one ACT table set
    (natural_log_exp_and_others) and Square is filler in every set, so
    the ACT table never reloads (v2 paid 4 table switches/iter, 5 us).
  * Output stored as f16 (host upcasts): halves store traffic; HBM
    floor drops from 36 to 34 MiB/core.
  * PSUM evac on ACT Copy (f32->f16).

Sharding: data-parallel over the 8192 tokens (B*T) across 8 NeuronCores.
"""

import os
import sys

import numpy as np

sys.path.insert(0, "/opt/trn_rl_repo")

N_BLOCKS, B, T, D = 7, 4, 2048, 1024
N_SRC = N_BLOCKS + 1          # 8 sources after appending `partial`
N_CORES = 8
TOK_TOTAL = B * T             # 8192
TOK_PER_CORE = TOK_TOTAL // N_CORES   # 1024
P = 128                       # SBUF partitions
QUADS = int(os.environ.get("KERNEL_QUADS", "2"))  # 128-token tiles per group
VBUFS = int(os.environ.get("KERNEL_VBUFS", "3"))  # v-tile pool depth
EPS = float(np.finfo(np.float32).eps)

# engine-split knobs (env-overridable for benchmarking)
# (Pool cannot take stats chunks: STT/TS accum_out fails codegen on Pool and
# gpsimd tensor_reduce is cross-partition only. Stats split DVE/ACT two-way.)
N_SQ_DVE = int(os.environ.get("KERNEL_N_SQ_DVE", "0"))   # sq chunks/group -> DVE
N_FOLD = int(os.environ.get("KERNEL_N_FOLD", "0"))       # sources folded on DVE STT
DIAG_ENG = os.environ.get("KERNEL_DIAG_ENG", "gpsimd")   # 'dve' | 'gpsimd'
EVAC_ACT = int(os.environ.get("KERNEL_EVAC_ACT", "0"))   # quads/group evac'd on ACT
OUT_F16 = os.environ.get("KERNEL_OUT_F16", "1") == "1"
LNEXP = os.environ.get("KERNEL_LNEXP", "1") == "1"       # rsqrt via exp(-ln/2)
# PSUM sinks for dummy outs hang the exec unit (NRT_EXEC_UNIT_UNRECOVERABLE,
# likely the PSUM-out + SBUF-accum_out encoding); keep off.
SINK_PSUM = os.environ.get("KERNEL_SINK_PSUM", "0") == "1"

_STATE: dict = {}


def _split_multi_waits(nc):
    """TPB instructions encode a single sem-wait; this walrus build refuses
    instructions carrying more (`Too many sync wait commands`). Split extra
    waits onto single-wait NoOps on the same engine, preserving per-engine
    program order (and therefore semantics)."""
    import concourse.mybir as mybir

    for fn in nc.m.functions:
        for blk in fn.blocks:
            insts = list(blk.instructions)
            out = []
            changed = False
            for ins in insts:
                si = ins.sync_info
                if si is not None and len(si.on_wait) > 1:
                    waits = list(si.on_wait)
                    for k, w in enumerate(waits[:-1]):
                        nop = mybir.InstNoOp(name=f"{ins.name}-sw{k}", ins=[], outs=[])
                        nop.engine = ins.engine
                        nop.sync_info = mybir.SyncInfo(on_wait=[w], on_update=[])
                        out.append(nop)
                    ins.sync_info = mybir.SyncInfo(
                        on_wait=[waits[-1]], on_update=list(si.on_update)
                    )
                    changed = True
                out.append(ins)
            if changed:
                blk.instructions = out
    return nc


def _spread(k: int, n: int = 32) -> set:
    """k indices spread evenly over [0, n)."""
    return {(j * n) // k for j in range(k)} if k > 0 else set()


def _build_nc(
    repeat: int = 1,
    loop: bool = True,
    n_sq_dve: int = N_SQ_DVE,
    n_fold: int = N_FOLD,
    diag_eng: str = DIAG_ENG,
    evac_act: int = EVAC_ACT,
    out_f16: bool = OUT_F16,
    lnexp: bool = LNEXP,
    quads: int = QUADS,
    vbufs: int = VBUFS,
    sink_psum: bool = SINK_PSUM,
):
    import concourse.bass as bass
    import concourse.mybir as mybir
    import concourse.tile as tile
    from contextlib import ExitStack

    f32 = mybir.dt.float32
    f16 = mybir.dt.float16
    Alu = mybir.AluOpType
    Act = mybir.ActivationFunctionType

    QUADS = quads                      # shadow module default inside builder
    N_G = TOK_PER_CORE // (P * QUADS)  # groups per core
    out_dt = f16 if out_f16 else f32

    nc = bass.Bass("TRN2", target_bir_lowering=False, debug=False)

    blocks_d = nc.dram_tensor(
        "blocks", [N_BLOCKS, TOK_PER_CORE, D], f32, kind="ExternalInput"
    )
    partial_d = nc.dram_tensor("partial", [TOK_PER_CORE, D], f32, kind="ExternalInput")
    wn_d = nc.dram_tensor("wnb", [P, D], f16, kind="ExternalInput")
    ident_d = nc.dram_tensor(
        "identb", [P, QUADS * N_SRC, P], f16, kind="ExternalInput"
    )
    out_d = nc.dram_tensor("out", [TOK_PER_CORE, D], out_dt, kind="ExternalOutput")

    # (g p q) d -> g p q d views: one DMA moves a full 2 MiB contiguous group
    # slice into a [128, 4, 1024] SBUF tile; each partition gets a contiguous
    # 16 KiB run (4 consecutive tokens), so descriptors are large and dense.
    # The in-group token permutation (partition-major) cancels between load
    # and store.
    bap = blocks_d.ap().rearrange("n (g p q) d -> n g p q d", p=P, q=QUADS)
    pap = partial_d.ap().rearrange("(g p q) d -> g p q d", p=P, q=QUADS)
    oap = out_d.ap().rearrange("(g p q) d -> g p q d", p=P, q=QUADS)

    fold_src = list(range(n_fold))                    # folded on DVE STT
    pe_src = list(range(n_fold, N_SRC))               # weighted sum on PE
    NQ8 = N_SRC * QUADS                               # stats width: 32

    dve_sq = _spread(n_sq_dve, N_SRC * QUADS)

    with tile.TileContext(nc) as tc, ExitStack() as ctx:
        const_pool = ctx.enter_context(tc.tile_pool(name="const", bufs=1))
        vpool = ctx.enter_context(tc.tile_pool(name="v", bufs=vbufs))
        scr_pool = ctx.enter_context(tc.tile_pool(name="scr", bufs=2))
        stat_pool = ctx.enter_context(tc.tile_pool(name="stat", bufs=2))
        diag_pool = ctx.enter_context(tc.tile_pool(name="diag", bufs=2))
        out_pool = ctx.enter_context(tc.tile_pool(name="outp", bufs=2))
        psum_pool = ctx.enter_context(tc.tile_pool(name="ps", bufs=1, space="PSUM"))

        wn_sb = const_pool.tile([P, D], f16, name="wn_sb")
        nc.sync.dma_start(wn_sb[:], wn_d.ap()[:, :])
        ident_sb = const_pool.tile([P, QUADS * N_SRC, P], f16, name="ident_sb")
        nc.sync.dma_start(ident_sb[:], ident_d.ap()[:, :, :])

        # PSUM sinks for the discarded elementwise outputs of accum ops:
        # keeps ~34 MB/iter of garbage writes off the SBUF ports (they were
        # slowing both the SDMA loads and the accum ops themselves).
        sinkV = sinkA = None
        if sink_psum:
            sinkV = psum_pool.tile([P, D], f32, tag="sinkV", name="sinkV", bufs=1)
            sinkA = psum_pool.tile([P, D], f32, tag="sinkA", name="sinkA", bufs=1)

        def emit_loads(g, r=0):
            # Loads get the most-negative priority: on the shared gpsimd
            # queue they must always outrank stats chunks and anything else,
            # so the SWDGE wire never sits behind a compute-dependent op in
            # the frozen per-engine order.
            v = []
            with tc.high_priority(offset=10**6):
                for n in range(N_BLOCKS):
                    vt = vpool.tile(
                        [P, QUADS, D], f16, tag=f"v{n}", name=f"v{n}_{g}_{r}"
                    )
                    nc.gpsimd.dma_start(vt[:], bap[n, g])
                    v.append(vt)
                vt = vpool.tile([P, QUADS, D], f16, tag="v7", name=f"v7_{g}_{r}")
                nc.gpsimd.dma_start(vt[:], pap[g])
                v.append(vt)
            return v

        def emit_stats_softmax(g, r, v):
            acc = out_pool.tile([P, QUADS, D], out_dt, tag="acc", name=f"acc_{g}_{r}")
            # ---- stats: s2 + dot, chunks balanced over ACT/DVE/Pool ----
            s2 = stat_pool.tile([P, NQ8], f32, tag="s2", name=f"s2_{g}")
            dot = stat_pool.tile([P, NQ8], f32, tag="dot", name=f"dot_{g}")
            if sink_psum:
                out_act, out_dve = sinkA[:], sinkV[:]
            else:
                dum_act = scr_pool.tile([P, 1], f16, tag="dumA", name=f"dumA_{g}")
                dum_dve = scr_pool.tile([P, 1], f16, tag="dumV", name=f"dumV_{g}")
                out_act = dum_act[:].broadcast_to((P, D))
                out_dve = dum_dve[:].broadcast_to((P, D))
            for n in range(N_SRC):
                # source-outer: the frozen per-engine order then consumes
                # sources in load-arrival order
                for q in range(QUADS):
                    i = n * QUADS + q
                    col = q * N_SRC + n
                    if i in dve_sq:
                        nc.vector.scalar_tensor_tensor(
                            out=out_dve,
                            in0=v[n][:, q, :],
                            scalar=1.0,
                            in1=v[n][:, q, :],
                            op0=Alu.mult,
                            op1=Alu.mult,
                            accum_out=s2[:, col : col + 1],
                        )
                    else:
                        nc.scalar.activation(
                            out_act,
                            v[n][:, q, :],
                            Act.Square,
                            accum_out=s2[:, col : col + 1],
                        )
                    nc.vector.scalar_tensor_tensor(
                        out=out_dve,
                        in0=v[n][:, q, :],
                        scalar=1.0,
                        in1=wn_sb[:],
                        op0=Alu.mult,
                        op1=Alu.mult,
                        accum_out=dot[:, col : col + 1],
                    )

            # ---- softmax over the 8 sources, batched over all quads ----
            # High priority (fixed offset): this chain must beat the NEXT
            # group's stats in the scheduler's per-engine heaps.
            hp = tc.high_priority()
            hp.__enter__()
            ms = stat_pool.tile([P, NQ8], f32, tag="ms", name=f"ms_{g}")
            nc.vector.tensor_scalar(ms[:], s2[:], 1.0 / D, EPS, Alu.mult, Alu.add)
            y = stat_pool.tile([P, NQ8], f32, tag="y", name=f"y_{g}")
            if lnexp:
                # y = 1/sqrt(ms) = exp(-0.5*ln(ms)): Ln+Exp live in ONE act
                # table set, so the ACT tables never reload across the run.
                t_ln = stat_pool.tile([P, NQ8], f32, tag="tln", name=f"tln_{g}")
                nc.scalar.activation(t_ln[:], ms[:], Act.Ln)
                nc.scalar.activation(y[:], t_ln[:], Act.Exp, scale=-0.5)
            else:
                rt = stat_pool.tile([P, NQ8], f32, tag="rt", name=f"rt_{g}")
                nc.scalar.activation(rt[:], ms[:], Act.Sqrt)
                nc.vector.reciprocal(y[:], rt[:])

            lg = stat_pool.tile([P, QUADS, N_SRC], f32, tag="lg", name=f"lg_{g}")
            nc.vector.tensor_tensor(
                lg[:].rearrange("p q n -> p (q n)"), dot[:], y[:], Alu.mult
            )
            nm = stat_pool.tile([P, QUADS, 1], f32, tag="nm", name=f"nm_{g}")
            nc.vector.tensor_reduce(
                nm[:], lg[:], axis=mybir.AxisListType.X, op=Alu.max, negate=True
            )
            lgs = stat_pool.tile([P, QUADS, N_SRC], f32, tag="lgs", name=f"lgs_{g}")
            nc.vector.tensor_tensor(
                lgs[:], lg[:], nm[:].broadcast_to((P, QUADS, N_SRC)), Alu.add
            )
            e = stat_pool.tile([P, QUADS, N_SRC], f32, tag="e", name=f"e_{g}")
            nc.scalar.activation(
                e[:].rearrange("p q n -> p (q n)"),
                lgs[:].rearrange("p q n -> p (q n)"),
                Act.Exp,
            )
            den = stat_pool.tile([P, QUADS, 1], f32, tag="den", name=f"den_{g}")
            nc.vector.tensor_reduce(den[:], e[:], axis=mybir.AxisListType.X, op=Alu.add)
            rcp = stat_pool.tile([P, QUADS, 1], f32, tag="rcp", name=f"rcp_{g}")
            nc.vector.reciprocal(rcp[:], den[:])
            wgt = stat_pool.tile([P, QUADS, N_SRC], f32, tag="wgt", name=f"wgt_{g}")
            nc.vector.tensor_tensor(
                wgt[:], e[:], rcp[:].broadcast_to((P, QUADS, N_SRC)), Alu.mult
            )
            return {"g": g, "r": r, "v": v, "wgt": wgt, "acc": acc, "hp": hp}

        def emit_wsum(st):
            g, r, v, wgt = st["g"], st["r"], st["v"], st["wgt"]
            acc, hp = st["acc"], st["hp"]

            # ---- weighted sum: PE diag matmuls + evac / DVE folds ----
            # All diag matrices of the group built in ONE op against a
            # MATERIALIZED replicated identity (broadcast-AP reads ran the
            # pool op at 2.7ns/elem and stalled concurrent DVE dots; real
            # in0 halves that):  dga[p, qn, j] = ident[p, j] * wgt[p, qn].
            half = D // 2
            dga = diag_pool.tile(
                [P, QUADS * N_SRC, P], f16, tag="dga", name=f"dga_{g}"
            )
            w_b = (
                wgt[:]
                .rearrange("p q n -> p (q n)")
                .broadcast_to((P, QUADS * N_SRC, P))
            )
            if diag_eng == "dve":
                nc.vector.tensor_tensor(dga[:], ident_sb[:], w_b, Alu.mult)
            else:
                nc.gpsimd.tensor_tensor(dga[:], ident_sb[:], w_b, Alu.mult)
            for q in range(QUADS):
                ps = psum_pool.tile(
                    [P, D], f32, tag=f"ps{q}", name=f"ps{q}_{g}",
                    bufs=(1 if (sink_psum or QUADS > 2) else 2),
                )
                for c in range(2):
                    cs = slice(c * half, (c + 1) * half)
                    for j, n in enumerate(pe_src):
                        nc.tensor.matmul(
                            ps[:, cs],
                            lhsT=dga[:, q * N_SRC + n, :],
                            rhs=v[n][:, q, cs],
                            start=(j == 0),
                            stop=(j == len(pe_src) - 1),
                        )
                if fold_src:
                    n0 = fold_src[0]
                    nc.vector.scalar_tensor_tensor(
                        out=acc[:, q, :],
                        in0=v[n0][:, q, :],
                        scalar=wgt[:, q, n0 : n0 + 1],
                        in1=ps[:],
                        op0=Alu.mult,
                        op1=Alu.add,
                    )
                    for n in fold_src[1:]:
                        nc.vector.scalar_tensor_tensor(
                            out=acc[:, q, :],
                            in0=v[n][:, q, :],
                            scalar=wgt[:, q, n : n + 1],
                            in1=acc[:, q, :],
                            op0=Alu.mult,
                            op1=Alu.add,
                        )
                elif q < evac_act:
                    nc.scalar.activation(acc[:, q, :], ps[:], Act.Copy)
                else:
                    nc.vector.tensor_scalar(
                        acc[:, q, :], ps[:], 1.0, 0.0, Alu.mult, Alu.add
                    )

            nc.sync.dma_start(oap[g], acc[:])
            hp.__exit__(None, None, None)

        def run_groups(rs):
            for r in rs:
                for g in range(N_G):
                    v = emit_loads(g, r)
                    emit_wsum(emit_stats_softmax(g, r, v))

        if repeat == 1 or not loop:
            run_groups(range(repeat))
        else:
            with tc.For_i(0, repeat, 1):
                run_groups([0])

    return _split_multi_waits(nc)


def _get_state():
    if "nc" not in _STATE:
        _STATE["nc"] = _build_nc()
    return _STATE["nc"]


def _prepare_in_maps(blocks, partial, norm_weight, w):
    blocks = np.asarray(blocks, dtype=np.float32)
    partial = np.asarray(partial, dtype=np.float32)
    norm_weight = np.asarray(norm_weight, dtype=np.float32)
    w = np.asarray(w, dtype=np.float32)

    wn = (w * norm_weight).astype(np.float32)
    wn_b = np.ascontiguousarray(np.broadcast_to(wn, (P, D)).astype(np.float16))
    ident = np.ascontiguousarray(
        np.broadcast_to(
            np.eye(P, dtype=np.float16)[:, None, :], (P, QUADS * N_SRC, P)
        )
    )

    blocks_f = blocks.reshape(N_BLOCKS, TOK_TOTAL, D)
    partial_f = partial.reshape(TOK_TOTAL, D)

    in_maps = []
    for c in range(N_CORES):
        sl = slice(c * TOK_PER_CORE, (c + 1) * TOK_PER_CORE)
        in_maps.append(
            {
                "blocks": np.ascontiguousarray(blocks_f[:, sl, :]),
                "partial": np.ascontiguousarray(partial_f[sl, :]),
                "wnb": wn_b,
                "identb": ident,
            }
        )
    return in_maps


def _run(inputs, trace=False, **kwargs):
    from concourse.bass_utils import run_bass_kernel_spmd

    nc = _get_state()
    in_maps = _prepare_in_maps(**inputs)
    bkr = run_bass_kernel_spmd(
        nc, in_maps, core_ids=list(range(N_CORES)), trace=trace, **kwargs
    )
    out = np.concatenate([bkr.results[c]["out"] for c in range(N_CORES)], axis=0)
    return out.reshape(B, T, D).astype(np.float32), bkr


def kernel(**inputs) -> np.ndarray:
    out, _ = _run(inputs, trace=False)
    return out


# revision 30
# speedup vs baseline: 1.0595x; 1.0160x over previous
"""Trainium2 Bass kernel for nn_BlockAttnRes (fused RMSNorm-softmax pooling), v3.

Reference computation (all fp32):
    V = concat([blocks, partial[None]], axis=0)          # (8, B, T, D)
    K = V * rsqrt(mean(V^2, -1) + eps) * norm_weight
    logits  = einsum('d,nbtd->nbt', w, K)
    weights = softmax(logits, axis=0)                    # over the 8 sources
    out     = einsum('nbt,nbtd->btd', weights, V)        # (B, T, D)

v3 design (vs v2 at ~130 us/iter, DVE+ACT ~98 us each, DMA ~104 us):
  * The 2x/4x DVE & ACT perf modes never engage for accum-bearing ops
    (HW-measured: any accum_out forces 1x mode), so each [128,1024]
    stats chunk costs ~1.3-1.4 us regardless of engine/dtype. Pool
    can't take reductions (no accum codegen, cross-partition-only
    reduce), so stats stay DVE/ACT, balanced: DVE = 64 dots + a few
    squares, ACT = remaining squares.
  * rsqrt via exp(-0.5*ln(ms)): Ln and Exp share one ACT table set
    (natural_log_exp_and_others) and Square is filler in every set, so
    the ACT table never reloads (v2 paid 4 table switches/iter, 5 us).
  * Output stored as f16 (host upcasts): halves store traffic; HBM
    floor drops from 36 to 34 MiB/core.
  * PSUM evac on ACT Copy (f32->f16).

Sharding: data-parallel over the 8192 tokens (B*T) across 8 NeuronCores.
"""

import os
import sys

import numpy as np

sys.path.insert(0, "/opt/trn_rl_repo")

N_BLOCKS, B, T, D = 7, 4, 2048, 1024
N_SRC = N_BLOCKS + 1          # 8 sources after appending `partial`
N_CORES = 8
TOK_TOTAL = B * T             # 8192
TOK_PER_CORE = TOK_TOTAL // N_CORES   # 1024
P = 128                       # SBUF partitions
QUADS = int(os.environ.get("KERNEL_QUADS", "2"))  # 128-token tiles per group
VBUFS = int(os.environ.get("KERNEL_VBUFS", "3"))  # v-tile pool depth
EPS = float(np.finfo(np.float32).eps)

# engine-split knobs (env-overridable for benchmarking)
# (Pool cannot take stats chunks: STT/TS accum_out fails codegen on Pool and
# gpsimd tensor_reduce is cross-partition only. Stats split DVE/ACT two-way.)
N_SQ_DVE = int(os.environ.get("KERNEL_N_SQ_DVE", "0"))   # sq chunks/group -> DVE
N_FOLD = int(os.environ.get("KERNEL_N_FOLD", "0"))       # sources folded on DVE STT
DIAG_ENG = os.environ.get("KERNEL_DIAG_ENG", "gpsimd")   # 'dve' | 'gpsimd'
EVAC_ACT = int(os.environ.get("KERNEL_EVAC_ACT", "0"))   # quads/group evac'd on ACT
OUT_F16 = os.environ.get("KERNEL_OUT_F16", "1") == "1"
LNEXP = os.environ.get("KERNEL_LNEXP", "1") == "1"       # rsqrt via exp(-ln/2)
# PSUM sinks for dummy outs hang the exec unit (NRT_EXEC_UNIT_UNRECOVERABLE,
# likely the PSUM-out + SBUF-accum_out encoding); keep off.
SINK_PSUM = os.environ.get("KERNEL_SINK_PSUM", "0") == "1"

_STATE: dict = {}


def _split_multi_waits(nc):
    """TPB instructions encode a single sem-wait; this walrus build refuses
    instructions carrying more (`Too many sync wait commands`). Split extra
    waits onto single-wait NoOps on the same engine, preserving per-engine
    program order (and therefore semantics)."""
    import concourse.mybir as mybir

    for fn in nc.m.functions:
        for blk in fn.blocks:
            insts = list(blk.instructions)
            out = []
            changed = False
            for ins in insts:
                si = ins.sync_info
                if si is not None and len(si.on_wait) > 1:
                    waits = list(si.on_wait)
                    for k, w in enumerate(waits[:-1]):
                        nop = mybir.InstNoOp(name=f"{ins.name}-sw{k}", ins=[], outs=[])
                        nop.engine = ins.engine
                        nop.sync_info = mybir.SyncInfo(on_wait=[w], on_update=[])
                        out.append(nop)
                    ins.sync_info = mybir.SyncInfo(
                        on_wait=[waits[-1]], on_update=list(si.on_update)
                    )
                    changed = True
                out.append(ins)
            if changed:
                blk.instructions = out
    return nc


def _spread(k: int, n: int = 32) -> set:
    """k indices spread evenly over [0, n)."""
    return {(j * n) // k for j in range(k)} if k > 0 else set()


def _build_nc(
    repeat: int = 1,
    loop: bool = True,
    n_sq_dve: int = N_SQ_DVE,
    n_fold: int = N_FOLD,
    diag_eng: str = DIAG_ENG,
    evac_act: int = EVAC_ACT,
    out_f16: bool = OUT_F16,
    lnexp: bool = LNEXP,
    quads: int = QUADS,
    vbufs: int = VBUFS,
    sink_psum: bool = SINK_PSUM,
):
    import concourse.bass as bass
    import concourse.mybir as mybir
    import concourse.tile as tile
    from contextlib import ExitStack

    f32 = mybir.dt.float32
    f16 = mybir.dt.float16
    Alu = mybir.AluOpType
    Act = mybir.ActivationFunctionType

    QUADS = quads                      # shadow module default inside builder
    N_G = TOK_PER_CORE // (P * QUADS)  # groups per core
    out_dt = f16 if out_f16 else f32

    nc = bass.Bass("TRN2", target_bir_lowering=False, debug=False)

    blocks_d = nc.dram_tensor(
        "blocks", [N_BLOCKS, TOK_PER_CORE, D], f32, kind="ExternalInput"
    )
    partial_d = nc.dram_tensor("partial", [TOK_PER_CORE, D], f32, kind="ExternalInput")
    wn_d = nc.dram_tensor("wnb", [P, D], f16, kind="ExternalInput")
    ident_d = nc.dram_tensor(
        "identb", [P, QUADS * N_SRC, P], f16, kind="ExternalInput"
    )
    out_d = nc.dram_tensor("out", [TOK_PER_CORE, D], out_dt, kind="ExternalOutput")

    # (g p q) d -> g p q d views: one DMA moves a full 2 MiB contiguous group
    # slice into a [128, 4, 1024] SBUF tile; each partition gets a contiguous
    # 16 KiB run (4 consecutive tokens), so descriptors are large and dense.
    # The in-group token permutation (partition-major) cancels between load
    # and store.
    bap = blocks_d.ap().rearrange("n (g p q) d -> n g p q d", p=P, q=QUADS)
    pap = partial_d.ap().rearrange("(g p q) d -> g p q d", p=P, q=QUADS)
    oap = out_d.ap().rearrange("(g p q) d -> g p q d", p=P, q=QUADS)

    fold_src = list(range(n_fold))                    # folded on DVE STT
    pe_src = list(range(n_fold, N_SRC))               # weighted sum on PE
    NQ8 = N_SRC * QUADS                               # stats width: 32

    dve_sq = _spread(n_sq_dve, N_SRC * QUADS)

    with tile.TileContext(nc) as tc, ExitStack() as ctx:
        const_pool = ctx.enter_context(tc.tile_pool(name="const", bufs=1))
        vpool = ctx.enter_context(tc.tile_pool(name="v", bufs=vbufs))
        scr_pool = ctx.enter_context(tc.tile_pool(name="scr", bufs=2))
        stat_pool = ctx.enter_context(tc.tile_pool(name="stat", bufs=2))
        diag_pool = ctx.enter_context(tc.tile_pool(name="diag", bufs=2))
        out_pool = ctx.enter_context(tc.tile_pool(name="outp", bufs=2))
        psum_pool = ctx.enter_context(tc.tile_pool(name="ps", bufs=1, space="PSUM"))

        wn_sb = const_pool.tile([P, D], f16, name="wn_sb")
        nc.sync.dma_start(wn_sb[:], wn_d.ap()[:, :])
        ident_sb = const_pool.tile([P, QUADS * N_SRC, P], f16, name="ident_sb")
        nc.sync.dma_start(ident_sb[:], ident_d.ap()[:, :, :])

        # PSUM sinks for the discarded elementwise outputs of accum ops:
        # keeps ~34 MB/iter of garbage writes off the SBUF ports (they were
        # slowing both the SDMA loads and the accum ops themselves).
        sinkV = sinkA = None
        if sink_psum:
            sinkV = psum_pool.tile([P, D], f32, tag="sinkV", name="sinkV", bufs=1)
            sinkA = psum_pool.tile([P, D], f32, tag="sinkA", name="sinkA", bufs=1)

        def emit_loads(g, r=0):
            # Loads get the most-negative priority: on the shared gpsimd
            # queue they must always outrank stats chunks and anything else,
            # so the SWDGE wire never sits behind a compute-dependent op in
            # the frozen per-engine order.
            v = []
            with tc.high_priority(offset=10**6):
                for n in range(N_BLOCKS):
                    vt = vpool.tile(
                        [P, QUADS, D], f16, tag=f"v{n}", name=f"v{n}_{g}_{r}"
                    )
                    nc.gpsimd.dma_start(vt[:], bap[n, g])
                    v.append(vt)
                vt = vpool.tile([P, QUADS, D], f16, tag="v7", name=f"v7_{g}_{r}")
                nc.gpsimd.dma_start(vt[:], pap[g])
                v.append(vt)
            return v

        def emit_stats_softmax(g, r, v):
            acc = out_pool.tile([P, QUADS, D], out_dt, tag="acc", name=f"acc_{g}_{r}")
            # ---- stats: s2 + dot, chunks balanced over ACT/DVE/Pool ----
            s2 = stat_pool.tile([P, NQ8], f32, tag="s2", name=f"s2_{g}")
            dot = stat_pool.tile([P, NQ8], f32, tag="dot", name=f"dot_{g}")
            if sink_psum:
                out_act, out_dve = sinkA[:], sinkV[:]
            else:
                dum_act = scr_pool.tile([P, 1], f16, tag="dumA", name=f"dumA_{g}")
                dum_dve = scr_pool.tile([P, 1], f16, tag="dumV", name=f"dumV_{g}")
                out_act = dum_act[:].broadcast_to((P, D))
                out_dve = dum_dve[:].broadcast_to((P, D))
            for n in range(N_SRC):
                # source-outer: the frozen per-engine order then consumes
                # sources in load-arrival order
                for q in range(QUADS):
                    i = n * QUADS + q
                    col = q * N_SRC + n
                    if i in dve_sq:
                        nc.vector.scalar_tensor_tensor(
                            out=out_dve,
                            in0=v[n][:, q, :],
                            scalar=1.0,
                            in1=v[n][:, q, :],
                            op0=Alu.mult,
                            op1=Alu.mult,
                            accum_out=s2[:, col : col + 1],
                        )
                    else:
                        nc.scalar.activation(
                            out_act,
                            v[n][:, q, :],
                            Act.Square,
                            accum_out=s2[:, col : col + 1],
                        )
                    nc.vector.scalar_tensor_tensor(
                        out=out_dve,
                        in0=v[n][:, q, :],
                        scalar=1.0,
                        in1=wn_sb[:],
                        op0=Alu.mult,
                        op1=Alu.mult,
                        accum_out=dot[:, col : col + 1],
                    )

            # ---- softmax over the 8 sources, batched over all quads ----
            # High priority (fixed offset): this chain must beat the NEXT
            # group's stats in the scheduler's per-engine heaps.
            hp = tc.high_priority()
            hp.__enter__()
            ms = stat_pool.tile([P, NQ8], f32, tag="ms", name=f"ms_{g}")
            nc.vector.tensor_scalar(ms[:], s2[:], 1.0 / D, EPS, Alu.mult, Alu.add)
            y = stat_pool.tile([P, NQ8], f32, tag="y", name=f"y_{g}")
            if lnexp:
                # y = 1/sqrt(ms) = exp(-0.5*ln(ms)): Ln+Exp live in ONE act
                # table set, so the ACT tables never reload across the run.
                t_ln = stat_pool.tile([P, NQ8], f32, tag="tln", name=f"tln_{g}")
                nc.scalar.activation(t_ln[:], ms[:], Act.Ln)
                nc.scalar.activation(y[:], t_ln[:], Act.Exp, scale=-0.5)
            else:
                rt = stat_pool.tile([P, NQ8], f32, tag="rt", name=f"rt_{g}")
                nc.scalar.activation(rt[:], ms[:], Act.Sqrt)
                nc.vector.reciprocal(y[:], rt[:])

            lg = stat_pool.tile([P, QUADS, N_SRC], f32, tag="lg", name=f"lg_{g}")
            nc.vector.tensor_tensor(
                lg[:].rearrange("p q n -> p (q n)"), dot[:], y[:], Alu.mult
            )
            nm = stat_pool.tile([P, QUADS, 1], f32, tag="nm", name=f"nm_{g}")
            nc.vector.tensor_reduce(
                nm[:], lg[:], axis=mybir.AxisListType.X, op=Alu.max, negate=True
            )
            lgs = stat_pool.tile([P, QUADS, N_SRC], f32, tag="lgs", name=f"lgs_{g}")
            nc.vector.tensor_tensor(
                lgs[:], lg[:], nm[:].broadcast_to((P, QUADS, N_SRC)), Alu.add
            )
            e = stat_pool.tile([P, QUADS, N_SRC], f32, tag="e", name=f"e_{g}")
            nc.scalar.activation(
                e[:].rearrange("p q n -> p (q n)"),
                lgs[:].rearrange("p q n -> p (q n)"),
                Act.Exp,
            )
            den = stat_pool.tile([P, QUADS, 1], f32, tag="den", name=f"den_{g}")
            nc.vector.tensor_reduce(den[:], e[:], axis=mybir.AxisListType.X, op=Alu.add)
            rcp = stat_pool.tile([P, QUADS, 1], f32, tag="rcp", name=f"rcp_{g}")
            nc.vector.reciprocal(rcp[:], den[:])
            wgt = stat_pool.tile([P, QUADS, N_SRC], f32, tag="wgt", name=f"wgt_{g}")
            nc.vector.tensor_tensor(
                wgt[:], e[:], rcp[:].broadcast_to((P, QUADS, N_SRC)), Alu.mult
            )
            return {"g": g, "r": r, "v": v, "wgt": wgt, "acc": acc, "hp": hp}

        def emit_wsum(st):
            g, r, v, wgt = st["g"], st["r"], st["v"], st["wgt"]
            acc, hp = st["acc"], st["hp"]

            # ---- weighted sum: PE diag matmuls + evac / DVE folds ----
            # All 8 diag matrices of a quad built in ONE fat op:
            # dga[p, n, j] = ident[p, j] * wgt[p, q, n]. (One-op-per-group
            # and materialized-ident variants both measured slower overall.)
            half = D // 2
            for q in range(QUADS):
                dga = diag_pool.tile(
                    [P, N_SRC, P], f16, tag=f"dga{q}", name=f"dga{q}_{g}"
                )
                i_b = ident_sb[:, 0:1, :].broadcast_to((P, N_SRC, P))
                w_b = wgt[:, q, :].broadcast_to((P, N_SRC, P))
                if diag_eng == "dve":
                    nc.vector.tensor_tensor(dga[:], i_b, w_b, Alu.mult)
                else:
                    nc.gpsimd.tensor_tensor(dga[:], i_b, w_b, Alu.mult)
                ps = psum_pool.tile(
                    [P, D], f32, tag=f"ps{q}", name=f"ps{q}_{g}",
                    bufs=(1 if (sink_psum or QUADS > 2) else 2),
                )
                for c in range(2):
                    cs = slice(c * half, (c + 1) * half)
                    for j, n in enumerate(pe_src):
                        nc.tensor.matmul(
                            ps[:, cs],
                            lhsT=dga[:, n, :],
                            rhs=v[n][:, q, cs],
                            start=(j == 0),
                            stop=(j == len(pe_src) - 1),
                        )
                if fold_src:
                    n0 = fold_src[0]
                    nc.vector.scalar_tensor_tensor(
                        out=acc[:, q, :],
                        in0=v[n0][:, q, :],
                        scalar=wgt[:, q, n0 : n0 + 1],
                        in1=ps[:],
                        op0=Alu.mult,
                        op1=Alu.add,
                    )
                    for n in fold_src[1:]:
                        nc.vector.scalar_tensor_tensor(
                            out=acc[:, q, :],
                            in0=v[n][:, q, :],
                            scalar=wgt[:, q, n : n + 1],
                            in1=acc[:, q, :],
                            op0=Alu.mult,
                            op1=Alu.add,
                        )
                elif q < evac_act:
                    nc.scalar.activation(acc[:, q, :], ps[:], Act.Copy)
                else:
                    nc.vector.tensor_scalar(
                        acc[:, q, :], ps[:], 1.0, 0.0, Alu.mult, Alu.add
                    )

            nc.sync.dma_start(oap[g], acc[:])
            hp.__exit__(None, None, None)

        def run_groups(rs):
            for r in rs:
                for g in range(N_G):
                    v = emit_loads(g, r)
                    emit_wsum(emit_stats_softmax(g, r, v))

        if repeat == 1 or not loop:
            run_groups(range(repeat))
        else:
            with tc.For_i(0, repeat, 1):
                run_groups([0])

    return _split_multi_waits(nc)


def _get_state():
    if "nc" not in _STATE:
        _STATE["nc"] = _build_nc()
    return _STATE["nc"]


def _prepare_in_maps(blocks, partial, norm_weight, w):
    blocks = np.asarray(blocks, dtype=np.float32)
    partial = np.asarray(partial, dtype=np.float32)
    norm_weight = np.asarray(norm_weight, dtype=np.float32)
    w = np.asarray(w, dtype=np.float32)

    wn = (w * norm_weight).astype(np.float32)
    wn_b = np.ascontiguousarray(np.broadcast_to(wn, (P, D)).astype(np.float16))
    ident = np.ascontiguousarray(
        np.broadcast_to(
            np.eye(P, dtype=np.float16)[:, None, :], (P, QUADS * N_SRC, P)
        )
    )

    blocks_f = blocks.reshape(N_BLOCKS, TOK_TOTAL, D)
    partial_f = partial.reshape(TOK_TOTAL, D)

    in_maps = []
    for c in range(N_CORES):
        sl = slice(c * TOK_PER_CORE, (c + 1) * TOK_PER_CORE)
        in_maps.append(
            {
                "blocks": np.ascontiguousarray(blocks_f[:, sl, :]),
                "partial": np.ascontiguousarray(partial_f[sl, :]),
                "wnb": wn_b,
                "identb": ident,
            }
        )
    return in_maps


def _run(inputs, trace=False, **kwargs):
    from concourse.bass_utils import run_bass_kernel_spmd

    nc = _get_state()
    in_maps = _prepare_in_maps(**inputs)
    bkr = run_bass_kernel_spmd(
        nc, in_maps, core_ids=list(range(N_CORES)), trace=trace, **kwargs
    )
    out = np.concatenate([bkr.results[c]["out"] for c in range(N_CORES)], axis=0)
    return out.reshape(B, T, D).astype(np.float32), bkr


def kernel(**inputs) -> np.ndarray:
    out, _ = _run(inputs, trace=False)
    return out
